# revision 1
# baseline (speedup 1.0000x reference)
"""AdaptiveProductHead retrieval scoring kernel for 8 TRN2 NeuronCores.

Strategy (corpus sharding, no collectives):
  - x_c [65536, 768] is split 8 ways along the corpus dim; each core scores
    its [512, 8192] block; the host concatenates. f32 output.
  - Host pre-transposes x_c/x_q to feature-major bf16 so the feature
    contraction (768) lands on SBUF partitions for matmuls; all small
    constants ship as one packed slab (single descriptor-gen on SP).
  - Algebraic reformulation (validated vs reference in numpy emulation and
    on hardware: max rel err 1.54e-2 vs the 2e-2 gate, dominated by bf16
    matmul rounding):
      * w0*dist_e = 2*w0 - se where se = 2*w0*(q_e.c_e) (w0 folded into
        query rows).
      * dist_s = arccos(x)^2: r = 1/(1+x) (DVE fast recip); t = sqrt(2r-1)
        (ACT, scale/bias); a2 = 1/(1+t) (fp16); v = arctan(1-2*a2) =
        arccos(x)/2 - pi/4 (ACT); q2 = w2*arccos^2 = (v*2*sqrt(w2) +
        pi/2*sqrt(w2))^2 (DVE two-scalar TSP + square).
      * w1*dist_h: lh = ln(sqrt(z)+sqrt(z+1)) = asinh(sqrt(z)) (2 ACT
        sqrts + Pool add + ACT ln); h2 = (lh*2*sqrt(w1))^2 (DVE).
      * Combine rides the se PSUM accumulation: -q2 and -h2 are added via
        negative-identity matmuls (PE has slack), deferred one group so PE
        order never blocks the next group's z/p matmuls; ot = se_ps - 2*w0
        splits across DVE TSP / ACT Identity (GPSIMD cannot read PSUM).
  - ACT table-set phases (sqrt -> arctan -> ln per group) are fenced, not
    chained: within a phase the static scheduler reorders freely.
  - Cost-model exec ~265 us (baseline kernel: 401 us); measured slope
    (dominated by ~0.3-1.3 ms/iter axon dispatch jitter) 0.79-1.7 ms
    across runs vs 1.31 ms baseline.
"""

import os
import sys
from contextlib import ExitStack

import numpy as np

sys.path.insert(0, "/opt/trn_rl_repo")

import ml_dtypes  # noqa: E402

import concourse.bass as bass  # noqa: E402
import concourse.tile as tile  # noqa: E402
from concourse import bacc, mybir  # noqa: E402

F32 = mybir.dt.float32
F16 = mybir.dt.float16
BF16 = mybir.dt.bfloat16
AX = mybir.AxisListType
OP = mybir.AluOpType
AF = mybir.ActivationFunctionType

D = 768
NQ = 512
NC = 65536
NCORES = 8
GROUP = 2048          # corpus columns processed per staged group
ST = 1024             # PSUM supertile width for score matmuls


def _build(shard: int):
    """Build the single-core SPMD graph for a corpus shard of `shard` cols."""
    assert shard % GROUP == 0
    n_groups = shard // GROUP
    nc = bacc.Bacc("TRN2", target_bir_lowering=False, debug=False,
                   num_devices=NCORES)

    xct = nc.dram_tensor("xct", [D, shard], BF16, kind="ExternalInput").ap()
    # packed [128, 6*NQ] query slab + packed [128, 7*96+128+128+4] const slab
    xqt = nc.dram_tensor("xqt", [128, 6 * NQ], BF16, kind="ExternalInput").ap()
    cslab = nc.dram_tensor("cslab", [128, 7 * 96 + 128 + 128 + 4], BF16,
                           kind="ExternalInput").ap()
    out = nc.dram_tensor("out", [NQ, shard], F32, kind="ExternalOutput").ap()

    with tile.TileContext(nc) as tc:
        _body(tc, xct, xqt, cslab, out, shard, n_groups)
    nc.compile()
    return nc


def _body(tc, xct, xqt, cslab, out, shard, n_groups):
    nc = tc.nc
    ctx = ExitStack()
    with ctx:
        _body_inner(ctx, tc, nc, xct, xqt, cslab, out, shard, n_groups)


def _body_inner(ctx, tc, nc, xct, xqt, cslab, out, shard, n_groups):
    sync = nc.sync
    from concourse.tile_rust import add_dep_helper
    # Table-set phase fencing: ACT runs its static schedule in order, so all
    # acts of table-phase k must precede all acts of phase k+1 — but WITHIN a
    # phase the scheduler may reorder freely (the old per-act chain forced
    # emission order and convoyed the whole pipeline). The first act of each
    # phase depends on every act of the previous phase; later acts of the
    # phase depend on that first act (transitively after phase k).
    _phase_prev = []     # instructions of the completed previous phase
    _phase_cur = []      # instructions of the current phase
    _phase_first = [None]

    def act_fence():
        nonlocal _phase_prev
        if _phase_cur:
            _phase_prev = list(_phase_cur)
            _phase_cur.clear()
        _phase_first[0] = None

    def act(out_ap, in_ap, func, **kw):
        inst = nc.scalar.activation(out_ap, in_ap, func, **kw)
        if _phase_first[0] is None:
            for prev in _phase_prev:
                add_dep_helper(inst.ins, prev, sync=False,
                               reason="act table-phase fence")
            _phase_first[0] = inst.ins
        else:
            add_dep_helper(inst.ins, _phase_first[0], sync=False,
                           reason="act table-phase order")
        _phase_cur.append(inst.ins)
        return inst
    # ---------------- pools ----------------
    consts = ctx.enter_context(tc.tile_pool(name="consts", bufs=1))
    qsmall = ctx.enter_context(tc.tile_pool(name="qsmall", bufs=1))
    # corpus prep
    xg_pool = ctx.enter_context(tc.tile_pool(name="xg", bufs=1))
    scratch_ps_pool = ctx.enter_context(tc.tile_pool(name="scratch_ps", bufs=1, space="PSUM"))
    praw_sb_pool = ctx.enter_context(tc.tile_pool(name="praw_sb", bufs=5))

    def scratch_f32():
        t = scratch_ps_pool.tile([128, 512], BF16, tag="s", name="scr")
        return t[:].bitcast(F32)

    def scratch_bf16():
        return scratch_ps_pool.tile([128, 512], BF16, tag="s", name="scr")
    norm_pool = ctx.enter_context(tc.tile_pool(name="norms", bufs=2))
    cproj_pool = ctx.enter_context(tc.tile_pool(name="cproj", bufs=2))
    cmaj_pool = ctx.enter_context(tc.tile_pool(name="cmaj", bufs=2))
    # main chain
    z_ps_pool = ctx.enter_context(tc.tile_pool(name="z_ps", bufs=2, space="PSUM"))
    p_ps_pool = ctx.enter_context(tc.tile_pool(name="p_ps", bufs=3, space="PSUM"))
    smbuf_pool = ctx.enter_context(tc.tile_pool(name="smbuf", bufs=1))
    abuf_pool = ctx.enter_context(tc.tile_pool(name="abuf", bufs=1))
    w2a2_pool = ctx.enter_context(tc.tile_pool(name="w2a2buf", bufs=1))
    tr32 = ctx.enter_context(tc.tile_pool(name="tr32", bufs=2))
    tr16 = ctx.enter_context(tc.tile_pool(name="tr16", bufs=2))
    outp = ctx.enter_context(tc.tile_pool(name="outp", bufs=3))

    # ---------------- constants (one slab DMA + one query DMA) ----------
    CS = 7 * 96 + 128 + 128 + 4
    cslab_sb = consts.tile([128, CS], BF16)
    sync.dma_start(out=cslab_sb[:], in_=cslab[:])
    wcat_sb = cslab_sb[:, 0:7 * 96]
    ident_sb = cslab_sb[:, 7 * 96:7 * 96 + 128]
    nident_sb = cslab_sb[:, 7 * 96 + 128:7 * 96 + 256]
    w2t_sb = cslab_sb[0:33, 7 * 96 + 256:7 * 96 + 260]
    xqt_sb = tr32.tile([128, 6 * NQ], BF16, tag="st_t", name="xqt_sb")
    sync.dma_start(out=xqt_sb[:], in_=xqt[:])
    ones1 = consts.tile([1, 128], BF16)
    nc.vector.memset(ones1[:], 1.0)

    qrows_sb = consts.tile([128, NQ], BF16)     # score-matmul query rows
    h1t_sb = consts.tile([33, NQ], BF16)
    nc.vector.memset(h1t_sb[32:33, :], 1.0)
    # per-query scalars, one column per q-chunk
    bm1 = consts.tile([128, 1], F32)
    nc.vector.memset(bm1[:], -1.0)
    w0x2 = consts.tile([128, 4], F32)           # 2*w0  (e-row scaling)
    w0n2 = consts.tile([128, 4], F32)           # -2*w0 (final bias)
    sw1x2 = consts.tile([128, 4], F32)          # 2*sqrt(w1)
    sw2x2 = consts.tile([128, 4], F32)          # 2*sqrt(w2)
    sw2pi = consts.tile([128, 4], F32)          # pi/2*sqrt(w2)

    # ---------------- early DMA for group 0 (overlaps query prep) -------
    xg0_early = []
    for k in range(6):
        t = xg_pool.tile([128, 1024], BF16, tag=f"xg{k}", name=f"xg{k}e")
        sync.dma_start(out=t[:], in_=xct[k * 128:(k + 1) * 128, 0:1024])
        xg0_early.append(t)

    # ---------------- query prep (stage-batched across q-chunks) --------
    qp_all = qsmall.tile([128, 4 * 96], F32, tag="qpall")
    for qc in range(4):
        qp_t = p_ps_pool.tile([128, 512], F32, tag="p", name="qp_ps")
        qp_ps = qp_t[:, 0:256]
        for k in range(6):
            nc.tensor.matmul(
                qp_ps[:, 0:96],
                lhsT=xqt_sb[:, k * NQ + qc * 128: k * NQ + (qc + 1) * 128],
                rhs=wcat_sb[:, k * 96:(k + 1) * 96],
                start=(k == 0), stop=False)
        nc.tensor.matmul(qp_ps[:, 0:96], lhsT=ones1[0:1, :],
                         rhs=wcat_sb[0:1, 6 * 96:7 * 96],
                         start=False, stop=True)
        nc.scalar.copy(qp_all[:, qc * 96:(qc + 1) * 96], qp_ps[:, 0:96])
    qp3 = qp_all[:].rearrange("p (q c) -> p q c", q=4)          # [128,4,96]
    sq_all = qsmall.tile([128, 256], F32, tag="qsq")
    nc.vector.tensor_mul(sq_all[:].rearrange("p (q c) -> p q c", q=4),
                         qp3[:, :, 0:64], qp3[:, :, 0:64])
    red = qsmall.tile([128, 16], F32, tag="qred")               # 4q x 4blk
    nc.vector.tensor_reduce(red[:],
                            sq_all[:].rearrange("p (b c) -> p b c", c=16),
                            axis=AX.X, op=OP.add)
    red3 = red[:].rearrange("p (q b) -> p q b", q=4)            # [128,4,4]
    ne2 = qsmall.tile([128, 4], F32, tag="qne2")
    nc.vector.tensor_add(ne2[:], red3[:, :, 0], red3[:, :, 1])
    rte = qsmall.tile([128, 4], F32, tag="qrte")
    act(rte[:], ne2[:], AF.Sqrt)                                # |e|
    rtsh = qsmall.tile([128, 8], F32, tag="qrtsh")              # (|s|,|h|) x4
    act(rtsh[:].rearrange("p (q b) -> p q b", q=4), red3[:, :, 2:4], AF.Sqrt)
    rtsh3 = rtsh[:].rearrange("p (q b) -> p q b", q=4)
    rce = qsmall.tile([128, 4], F32, tag="qrce")
    nc.vector.reciprocal_approx_fast(rce[:], rte[:])            # 1/|e|
    rcsh = qsmall.tile([128, 8], F32, tag="qrcsh")
    nc.vector.reciprocal_approx_fast(rcsh[:], rtsh[:])          # 1/|s|,1/|h|
    rcsh3 = rcsh[:].rearrange("p (q b) -> p q b", q=4)
    act_fence()
    th = qsmall.tile([128, 4], F32, tag="qth")
    act(th[:], rtsh3[:, :, 1], AF.Tanh)                         # tanh(|h|)
    xn = qsmall.tile([128, 4], F32, tag="qxn")
    nc.vector.tensor_mul(xn[:], th[:], th[:])
    omx = qsmall.tile([128, 4], F32, tag="qomx")
    nc.vector.tensor_scalar(omx[:], xn[:], -1.0, 1.0, OP.mult, OP.add)
    ib = qsmall.tile([128, 4], F32, tag="qib")
    nc.vector.reciprocal_approx_fast(ib[:], omx[:])
    f_h = qsmall.tile([128, 4], F32, tag="qfh")
    nc.vector.tensor_mul(f_h[:], th[:], rcsh3[:, :, 1])         # tanh(n)/n
    fh2 = qsmall.tile([128, 4], F32, tag="qfh2")
    nc.vector.tensor_mul(fh2[:], f_h[:], ib[:])
    nc.vector.tensor_scalar_mul(fh2[:], fh2[:], -2.0)
    xnib = qsmall.tile([128, 4], F32, tag="qxnib")
    nc.vector.tensor_mul(xnib[:], xn[:], ib[:])
    # MLP: relu -> per-qc transpose -> matmul, then batched softplus
    h1_all = qsmall.tile([128, 128], BF16, tag="qh1")
    nc.vector.tensor_relu(h1_all[:].rearrange("p (q c) -> p q c", q=4),
                          qp3[:, :, 64:96])
    wpre_t = z_ps_pool.tile([128, ST], F32, tag="z", name="wpre")
    wpre = wpre_t[:, 0:256]
    for qc in range(4):
        h1tp_t = p_ps_pool.tile([128, 512], F32, tag="p", name="h1tp")
        h1tp = h1tp_t[:].bitcast(BF16)
        nc.tensor.transpose(h1tp[0:32, 0:128],
                            h1_all[:, qc * 32:(qc + 1) * 32], ident_sb)
        nc.vector.tensor_copy(h1t_sb[0:32, qc * 128:(qc + 1) * 128],
                              h1tp[0:32, 0:128])
        nc.tensor.matmul(wpre[:, qc * 4:(qc + 1) * 4],
                         lhsT=h1t_sb[0:33, qc * 128:(qc + 1) * 128],
                         rhs=w2t_sb, start=True, stop=True,
                         tile_position=(0, 0))
    act_fence()
    wex = qsmall.tile([128, 16], F32, tag="qwex")
    act(wex[:], wpre[:, 0:16], AF.Exp)
    wts = qsmall.tile([128, 16], F32, tag="qwts")
    act(wts[:], wex[:], AF.Ln, bias=1.0)                        # softplus
    wts3 = wts[:].rearrange("p (q c) -> p q c", q=4)
    nc.vector.tensor_scalar_mul(w0x2[:], wts3[:, :, 0], 2.0)
    nc.vector.tensor_scalar_mul(w0n2[:], wts3[:, :, 0], -2.0)
    # sqrt(w1), sqrt(w2) for the Square-based weighting
    act_fence()
    sqw = qsmall.tile([128, 8], F32, tag="qsqw")
    act(sqw[:].rearrange("p (q b) -> p q b", q=4), wts3[:, :, 1:3], AF.Sqrt)
    sqw3 = sqw[:].rearrange("p (q b) -> p q b", q=4)            # [128,4,2]
    nc.vector.tensor_scalar_mul(sw1x2[:], sqw3[:, :, 0], 2.0)
    nc.vector.tensor_scalar_mul(sw2x2[:], sqw3[:, :, 1], 2.0)
    nc.vector.tensor_scalar_mul(sw2pi[:], sqw3[:, :, 1], float(np.pi / 2))
    ce = qsmall.tile([128, 4], F32, tag="qce")
    nc.vector.tensor_mul(ce[:], rce[:], w0x2[:])
    # assemble q_all (bf16) for all 4 chunks, then transpose into qrows
    qall = qsmall.tile([128, 512], BF16, tag="qall")
    nc.vector.memset(qall[:], 0.0)
    qa3 = qall[:].rearrange("p (q c) -> p q c", q=4)            # [128,4,128]
    def qbc(sc):
        return sc[:].unsqueeze(2)                               # [128,4,1]
    b0, b1 = bass.broadcast_tensor_aps(qp3[:, :, 0:32], qbc(ce))
    nc.vector.tensor_tensor(qa3[:, :, 0:32], b0, b1, OP.mult)
    b0, b1 = bass.broadcast_tensor_aps(qp3[:, :, 32:48], qbc(rcsh3[:, :, 0]))
    nc.vector.tensor_tensor(qa3[:, :, 32:48], b0, b1, OP.mult)
    b0, b1 = bass.broadcast_tensor_aps(qp3[:, :, 48:64], qbc(fh2))
    nc.vector.tensor_tensor(qa3[:, :, 64:80], b0, b1, OP.mult)
    nc.vector.memset(qa3[:, :, 48:49], 1.0)
    nc.vector.tensor_copy(qa3[:, :, 80:81], qbc(xnib))
    nc.vector.tensor_copy(qa3[:, :, 81:82], qbc(ib))
    for qc in range(4):
        qtp_t = p_ps_pool.tile([128, 512], F32, tag="p", name="qtp")
        qtp = qtp_t[:].bitcast(BF16)
        nc.tensor.transpose(qtp[:, 0:128],
                            qall[:, qc * 128:(qc + 1) * 128], ident_sb)
        nc.vector.tensor_copy(qrows_sb[:, qc * 128:(qc + 1) * 128],
                              qtp[:, 0:128])

    # ---------------- corpus prep (split into table-set phases) ----------
    def stage_xg(g):
        """Emit the corpus-slice DMAs for group g (both halves) early so the
        SP sequencer isn't head-of-line blocked behind output-store waits."""
        base = g * GROUP
        halves = []
        for half in range(2):
            if g == 0 and half == 0:
                halves.append(xg0_early)
                continue
            xg = []
            for k in range(6):
                t = xg_pool.tile([128, 1024], BF16, tag=f"xg{k}",
                                 name=f"xg{k}h{half}")
                sync.dma_start(
                    out=t[:],
                    in_=xct[k * 128:(k + 1) * 128,
                            base + half * 1024: base + (half + 1) * 1024])
                xg.append(t)
            halves.append(xg)
        return halves

    def prep_a(g, xg_halves):
        """Projection matmuls + squared-norm reduce + sqrt-set norms."""
        red_g = norm_pool.tile([128, 64], F32, tag="red")       # 16c x 4t
        praw_sbs = []
        for half in range(2):
            xg = xg_halves[half]
            for pk in range(half * 2, half * 2 + 2):  # 2 packs per half
                praw_ps = scratch_f32()
                for j in range(4):                    # chunk within pack
                    cc = (pk - half * 2) * 4 + j      # chunk within half
                    sl = praw_ps[:, j * 64:(j + 1) * 64]
                    for k in range(6):
                        nc.tensor.matmul(
                            sl, lhsT=xg[k][:, cc * 128:(cc + 1) * 128],
                            rhs=wcat_sb[:, k * 96: k * 96 + 64],
                            start=(k == 0), stop=False)
                    nc.tensor.matmul(sl, lhsT=ones1[0:1, :],
                                     rhs=wcat_sb[0:1, 6 * 96: 6 * 96 + 64],
                                     start=False, stop=True)
                praw_sb = praw_sb_pool.tile([128, 256], F32, tag="praw_sb")
                nc.vector.tensor_copy(praw_sb[:], praw_ps[:])
                praw_sbs.append(praw_sb)
                sq = praw_sb_pool.tile([128, 256], F32, tag="sqp", bufs=1)
                nc.vector.tensor_mul(sq[:], praw_sb[:], praw_sb[:])
                sq3 = sq[:].rearrange("p (c d) -> p c d", d=16)  # [128,16,16]
                nc.vector.tensor_reduce(red_g[:, pk * 16:(pk + 1) * 16],
                                        sq3, axis=AX.X, op=OP.add)
        red3 = red_g[:].rearrange("p (c t) -> p c t", t=4)      # [128,16,4]
        ne2 = norm_pool.tile([128, 16], F32, tag="ne2")
        nc.vector.tensor_add(ne2[:], red3[:, :, 0], red3[:, :, 1])
        rt_es = norm_pool.tile([128, 32], F32, tag="rt_es")     # |e| then |s|
        act(rt_es[:, 0:16], ne2[:], AF.Sqrt)
        act(rt_es[:, 16:32], red3[:, :, 2], AF.Sqrt)
        rth = norm_pool.tile([128, 16], F32, tag="rth")
        act(rth[:], red3[:, :, 3], AF.Sqrt)
        return dict(praw_sbs=praw_sbs, rt_es=rt_es, rth=rth)

    def prep_mid(pc):
        """Tanh of the hyperbolic norms (sigmoid table set). Runs inside
        the sigmoid phase opened by arctan_stage (no fence here)."""
        th = norm_pool.tile([128, 16], F32, tag="cth")
        act(th[:], pc["rth"][:], AF.Tanh)
        pc["th"] = th

    def prep_b(pc):
        """Scale factors + c-major assembly + PE transposes -> cproj."""
        rt_es, rth, th = pc["rt_es"], pc["rth"], pc["th"]
        cproj = cproj_pool.tile([128, GROUP], BF16, tag="cproj")
        fes = norm_pool.tile([128, 32], F32, tag="fes")
        nc.vector.reciprocal_approx_fast(fes[:], rt_es[:])      # 1/|e|, 1/|s|
        rcth = norm_pool.tile([128, 16], F32, tag="rcth")
        nc.vector.reciprocal_approx_fast(rcth[:], rth[:])
        f_h = norm_pool.tile([128, 16], F32, tag="cfh")
        nc.vector.tensor_mul(f_h[:], th[:], rcth[:])
        yn = norm_pool.tile([128, 16], F32, tag="cyn")
        nc.vector.tensor_mul(yn[:], th[:], th[:])
        omy = norm_pool.tile([128, 16], F32, tag="comy")
        nc.vector.tensor_scalar(omy[:], yn[:], -1.0, 1.0, OP.mult, OP.add)
        iy = norm_pool.tile([128, 16], F32, tag="ciy")
        nc.vector.reciprocal_approx_fast(iy[:], omy[:])
        fhiy = norm_pool.tile([128, 16], F32, tag="cfhiy")
        nc.vector.tensor_mul(fhiy[:], f_h[:], iy[:])
        yniy = norm_pool.tile([128, 16], F32, tag="cyniy")
        nc.vector.tensor_mul(yniy[:], yn[:], iy[:])
        for pk in range(4):
            praw_sb = pc["praw_sbs"][pk]
            p3 = praw_sb[:].rearrange("p (c f) -> p c f", c=4)  # [128,4,64]
            cm = cmaj_pool.tile([128, 512], BF16, tag="cmaj")
            nc.gpsimd.memset(cm[:], 0.0)
            c3 = cm[:].rearrange("p (c f) -> p c f", c=4)       # [128,4,128]
            def bc(sc):
                return sc[:, pk * 4:(pk + 1) * 4].unsqueeze(2)  # [128,4,1]
            b0, b1 = bass.broadcast_tensor_aps(p3[:, :, 0:32], bc(fes[:, 0:16]))
            nc.gpsimd.tensor_tensor(c3[:, :, 0:32], b0, b1, OP.mult)
            b0, b1 = bass.broadcast_tensor_aps(p3[:, :, 32:48], bc(fes[:, 16:32]))
            nc.gpsimd.tensor_tensor(c3[:, :, 32:48], b0, b1, OP.mult)
            b0, b1 = bass.broadcast_tensor_aps(p3[:, :, 48:64], bc(fhiy))
            nc.gpsimd.tensor_tensor(c3[:, :, 64:80], b0, b1, OP.mult)
            nc.vector.memset(c3[:, :, 48:49], 1.0)
            nc.gpsimd.tensor_copy(c3[:, :, 80:81], bc(iy))
            nc.gpsimd.tensor_copy(c3[:, :, 81:82], bc(yniy))
            tp = scratch_bf16()
            for j in range(4):
                nc.tensor.transpose(tp[:, j * 128:(j + 1) * 128],
                                    cm[:, j * 128:(j + 1) * 128], ident_sb)
            nc.vector.tensor_copy(
                cproj[:, pk * 512:(pk + 1) * 512].bitcast(mybir.dt.uint32),
                tp[:].bitcast(mybir.dt.uint32))
        return cproj

    # ---------------- main chain stages ----------------
    def sqrt_stage(g, cproj):
        """Sqrt-table phase: z/p matmuls, r=1/p, t=sqrt(2r-1), a2=1/(1+t),
        sz=sqrt(z), sz1=sqrt(z+1), sm=sz+sz1 (Pool)."""
        act_fence()
        sm = [smbuf_pool.tile([128, GROUP], F32, tag=f"sm{qc}", name=f"sm{qc}")
              for qc in range(4)]
        # a2 = 1/(1+t) in fp16: [0.15, 1] needs only ~3 decimal digits for
        # the arctan argument; halves the inter-phase SBUF footprint.
        ab32 = [abuf_pool.tile([128, GROUP], F16, tag=f"ab32_{qc}",
                               name=f"ab32_{qc}") for qc in range(4)]
        # Pass 1: all matmuls + p-reciprocals + sqrt(z)/sqrt(z+1)/sm for all
        # q-chunks. The recips sit early in DVE program order so the rect
        # tiles are ready when ACT reaches the t-passes, even when DVE enters
        # the group with a backlog from the previous group's combine.
        rects = []
        for qc in range(4):
            rect = tr32.tile([128, GROUP], F32, tag="rect", bufs=3)
            for st in range(GROUP // ST):
                lo = st * ST
                sl = slice(lo, lo + ST)
                z_ps = z_ps_pool.tile([128, ST], F32, tag="z")
                for h in range(2):
                    cs = slice(lo + h * 512, lo + (h + 1) * 512)
                    p_ps = p_ps_pool.tile([128, 512], F32, tag="p", name="p_ps")
                    nc.tensor.matmul(z_ps[:, h * 512:(h + 1) * 512],
                                     lhsT=qrows_sb[64:82, qc * 128:(qc + 1) * 128],
                                     rhs=cproj[64:82, cs],
                                     tile_position=(64, 0), start=True, stop=True)
                    nc.tensor.matmul(p_ps[:],
                                     lhsT=qrows_sb[32:49, qc * 128:(qc + 1) * 128],
                                     rhs=cproj[32:49, cs],
                                     tile_position=(32, 0), start=True, stop=True)
                    nc.vector.reciprocal_approx_fast(rect[:, cs], p_ps[:])
                szt = tr32.tile([128, ST], F32, tag="szt", bufs=2)
                s1zt = tr32.tile([128, ST], F32, tag="s1zt", bufs=2)
                act(szt[:], z_ps[:], AF.Sqrt)
                act(s1zt[:], z_ps[:], AF.Sqrt, bias=1.0)
                nc.gpsimd.tensor_add(sm[qc][:, sl], szt[:], s1zt[:])
            rects.append(rect)
        # Pass 2: t = sqrt(2r-1), then a2 = 1/(1+t) in fp16.
        from concourse.dve_ops import (RECIP_APPROX_FAST_CONSTS,
                                       RECIPROCAL_APPROX_FAST)
        c = RECIP_APPROX_FAST_CONSTS
        for qc in range(4):
            st_t = tr32.tile([128, GROUP], F32, tag="st_t", bufs=2)
            act(st_t[:], rects[qc][:], AF.Sqrt, bias=bm1[:], scale=2.0)
            nc.vector.tensor_scalar_add(st_t[:], st_t[:], 1.0)   # in-place t+1
            # recip with fp16 output (the fp32-bit-trick constraint is on the
            # input; the DVE output stage converts)
            nc.vector._custom_dve(RECIPROCAL_APPROX_FAST, out=ab32[qc][:],
                                  in0=st_t[:], s0=c["s0"], s1=c["s1"],
                                  imm2=c["imm2"])
        return sm, ab32

    def arctan_stage(g, ab32):
        """Sigmoid-table phase: v = arctan(1-2*a2) = arccos(x)/2 - pi/4.
        (The Square weighting runs in the ln phase — square is in every
        table set — so this ACT-only phase is short.)"""
        act_fence()
        q2 = [w2a2_pool.tile([128, GROUP], BF16, tag=f"q2_{qc}",
                             name=f"q2_{qc}") for qc in range(4)]
        for qc in range(4):
            v = tr16.tile([128, GROUP], BF16, tag="v", bufs=2)
            act(v[:], ab32[qc][:], AF.Arctan, bias=1.0, scale=-2.0)
            # q2 = (v*2*sqrt(w2) + pi/2*sqrt(w2))^2 on DVE (bf16 4x TSP +
            # in-place square) — keeps the short sigmoid ACT phase short.
            nc.vector.tensor_scalar(q2[qc][:], v[:], sw2x2[:, qc:qc + 1],
                                    sw2pi[:, qc:qc + 1], OP.mult, OP.add)
            nc.vector.tensor_mul(q2[qc][:], q2[qc][:], q2[qc][:])
        return q2

    def ln_stage(g, sm):
        """Ln-table phase: lh=ln(sm), h2=(2*sqrt(w1)*lh)^2 (DVE, squared in
        place). Produces the per-qc h2 tiles for the deferred combine."""
        act_fence()
        h2s = []
        for qc in range(4):
            # bf16 lh: costs ~+0.3% worst-case error (emulated 1.65% vs the
            # 2% gate) and unlocks the 4x DVE mode on the ph scaling pass
            lh = tr16.tile([128, GROUP], BF16, tag="vlh16", bufs=2)
            act(lh[:], sm[qc][:], AF.Ln)
            h2 = tr16.tile([128, GROUP], BF16, tag=f"h2_{qc}", bufs=1)
            nc.vector.tensor_single_scalar(h2[:], lh[:],
                                           sw1x2[:, qc:qc + 1], OP.mult)
            nc.vector.tensor_mul(h2[:], h2[:], h2[:])    # in-place square
            h2s.append(h2)
        return h2s

    def combine_group(g, cproj, q2, h2s):
        """Deferred combine for group g (emitted after group g+1's z/p
        matmuls so PE order never blocks the next group's sqrt phase):
        se PSUM accumulation absorbs -q2 and -h2 via negative-identity
        matmuls; ot = se_ps - 2*w0 is a Pool TSP (latency-tolerant: it only
        feeds the output DMA)."""
        base = g * GROUP
        for qc in range(4):
            for st in range(GROUP // ST):
                lo = st * ST
                ot = outp.tile([128, ST], F32, tag="ot")
                for h in range(2):
                    cs = slice(lo + h * 512, lo + (h + 1) * 512)
                    hs = slice(h * 512, (h + 1) * 512)
                    se_ps = p_ps_pool.tile([128, 512], F32, tag="p",
                                           name="se_ps")
                    nc.tensor.matmul(se_ps[:],
                                     lhsT=qrows_sb[0:32, qc * 128:(qc + 1) * 128],
                                     rhs=cproj[0:32, cs],
                                     tile_position=(0, 0), start=True,
                                     stop=False, skip_group_check=True)
                    nc.tensor.matmul(se_ps[:], lhsT=nident_sb,
                                     rhs=q2[qc][:, cs], tile_position=(0, 0),
                                     start=False, stop=False,
                                     skip_group_check=True)
                    nc.tensor.matmul(se_ps[:], lhsT=nident_sb,
                                     rhs=h2s[qc][:, cs], tile_position=(0, 0),
                                     start=False, stop=True,
                                     skip_group_check=True)
                    if h == 0:
                        nc.vector.tensor_single_scalar(ot[:, hs], se_ps[:],
                                                       w0n2[:, qc:qc + 1],
                                                       OP.add)
                    else:
                        # Identity is in every act table set: no fence needed
                        nc.scalar.activation(ot[:, hs], se_ps[:], AF.Identity,
                                             bias=w0n2[:, qc:qc + 1])
                sync.dma_start(
                    out=out[qc * 128:(qc + 1) * 128, base + lo: base + lo + ST],
                    in_=ot[:])

    # ---------------- top-level schedule ----------------
    # stage_xg(g) is emitted right after prep_a(g-1) so its WAR wait (on the
    # previous group's projection matmuls) is short when it reaches the head
    # of the SP queue — and it always precedes the output-store DMAs of the
    # group before it, keeping corpus loads ahead of store-side waits.
    xg_h = stage_xg(0)
    pc = prep_a(0, xg_h)
    xg_nxt = stage_xg(1) if n_groups > 1 else None
    prep_mid(pc)
    cproj = prep_b(pc)
    pend = None          # (g, cproj, q2, h2s) awaiting deferred combine
    for g in range(n_groups):
        sm, ab32 = sqrt_stage(g, cproj)                  # sqrt set
        if pend is not None:
            combine_group(*pend)                         # prev group: PE+Pool
        pc_n = prep_a(g + 1, xg_nxt) if g + 1 < n_groups else None  # sqrt set
        xg_nxt = stage_xg(g + 2) if g + 2 < n_groups else None
        q2 = arctan_stage(g, ab32)                       # sigmoid set
        if pc_n is not None:
            prep_mid(pc_n)                               # sigmoid set
        cproj_n = prep_b(pc_n) if pc_n is not None else None
        h2s = ln_stage(g, sm)                            # ln set
        pend = (g, cproj, q2, h2s)
        cproj = cproj_n
    combine_group(*pend)


# ---------------------------------------------------------------------------
# host-side entry point
# ---------------------------------------------------------------------------
_CACHE = {}
_LAST_RESULTS = None


def _prep_host_inputs(x_q, x_c, We, be, Wh, bh, Ws, bs, scale_h, W1, b1, W2, b2):
    bf = ml_dtypes.bfloat16
    sh = np.float32(scale_h)
    W_all = np.concatenate([We, Ws, sh * Wh, W1], axis=0).astype(np.float32)  # [96,768]
    b_all = np.concatenate([be, bs, sh * bh, b1], axis=0).astype(np.float32)  # [96]
    wcat = np.zeros((7 * 128, 96), np.float32)
    wcat[:768, :] = W_all.T
    wcat[768, :] = b_all
    w2t = np.zeros((33, 4), np.float32)
    w2t[:32, :3] = W2.T
    w2t[32, :3] = b2
    xqt = np.ascontiguousarray(x_q.T)
    xct = np.ascontiguousarray(x_c.T)
    # pack xqt [768, 512] -> [128, 6*512] partition-major
    xqt_p = np.concatenate([xqt[k * 128:(k + 1) * 128, :] for k in range(6)],
                           axis=1)
    # const slab: wcat [7*128, 96] -> [128, 7*96], ident, nident, w2t-pad
    wcat_p = np.concatenate([wcat[k * 128:(k + 1) * 128, :] for k in range(7)],
                            axis=1)
    w2t_pad = np.zeros((128, 4), np.float32)
    w2t_pad[:33, :] = w2t
    cslab = np.concatenate([
        wcat_p, np.eye(128, dtype=np.float32),
        -np.eye(128, dtype=np.float32), w2t_pad], axis=1)
    return {
        "xqt": xqt_p.astype(bf),
        "xct": xct.astype(bf),
        "cslab": cslab.astype(bf),
    }


def _ensure_trn_backend():
    """Make sure jax sees the 8 axon TRN cores even if another part of the
    process pinned jax to cpu first."""
    import jax
    try:
        devs = jax.devices()
        if len(devs) >= NCORES and devs[0].platform != "cpu":
            return
    except Exception:
        pass
    try:
        jax.config.update("jax_platforms", "axon")
        import jax.extend.backend
        jax.extend.backend.clear_backends()
        devs = jax.devices()
        assert len(devs) >= NCORES, devs
    except Exception as e:
        print("kernel: TRN backend re-init failed:", repr(e))


def kernel(x_q, x_c, We, be, Wh, bh, Ws, bs, scale_h, W1, b1, W2, b2):
    from concourse.bass_utils import run_bass_kernel_spmd

    _ensure_trn_backend()

    n_c = x_c.shape[0]
    shard = n_c // NCORES
    host = _prep_host_inputs(x_q, x_c, We, be, Wh, bh, Ws, bs, scale_h,
                             W1, b1, W2, b2)
    if shard not in _CACHE:
        _CACHE[shard] = _build(shard)
    nc = _CACHE[shard]
    in_maps = []
    for c in range(NCORES):
        m = {k: v for k, v in host.items() if k != "xct"}
        m["xct"] = np.ascontiguousarray(
            host["xct"][:, c * shard:(c + 1) * shard])
        in_maps.append(m)
    global _LAST_RESULTS
    trace = bool(int(os.environ.get("KBENCH_TRACE", "0")))
    res = run_bass_kernel_spmd(nc, in_maps, core_ids=list(range(NCORES)),
                               trace=trace)
    _LAST_RESULTS = res
    outs = [np.asarray(res.results[c]["out"]).astype(np.float32)
            for c in range(NCORES)]
    return np.concatenate(outs, axis=1)


if __name__ == "__main__":
    # smoke-build at small shard
    nc = _build(GROUP)
    print("build ok")


def _pjrt_timed(nc, in_maps, iters):
    """Time `iters` back-to-back NEFF executions with device-resident inputs.
    Returns (t_total_seconds, per_iter_overhead_estimate)."""
    import time as _time

    import jax
    from jax.experimental.shard_map import shard_map
    from jax.sharding import Mesh, PartitionSpec, NamedSharding

    from concourse import bass2jax as b2j
    from concourse import mybir as _mb

    b2j.install_neuronx_cc_hook()
    partition_name = (nc.partition_id_tensor.name
                      if nc.partition_id_tensor else None)
    in_names, out_names, out_avals, zero_outs = [], [], [], []
    for alloc in nc.m.functions[0].allocations:
        if not isinstance(alloc, _mb.MemoryLocationSet):
            continue
        name = alloc.memorylocations[0].name
        if alloc.kind == "ExternalInput":
            if name != partition_name:
                in_names.append(name)
        elif alloc.kind == "ExternalOutput":
            shape = tuple(alloc.tensor_shape)
            dtype = _mb.dt.np(alloc.dtype)
            out_avals.append(jax.core.ShapedArray(shape, dtype))
            zero_outs.append(np.zeros(shape, dtype))
            out_names.append(name)
    n_params = len(in_names)
    n_outs = len(out_avals)
    in_names = in_names + out_names
    if partition_name is not None:
        in_names.append(partition_name)

    def _per_core(m):
        return [np.asarray(m[name]) for name in in_names[:n_params]]

    def _body(*args):
        operands = list(args)
        if partition_name is not None:
            operands.append(b2j.partition_id_tensor())
        outs = b2j._bass_exec_p.bind(
            *operands,
            out_avals=tuple(out_avals),
            in_names=tuple(in_names),
            out_names=tuple(out_names),
            lowering_input_output_aliases=(),
            sim_require_finite=True,
            sim_require_nnan=True,
            nc=nc,
        )
        return tuple(outs)

    n_cores = len(in_maps)
    devices = jax.devices()[:n_cores]
    mesh = Mesh(np.asarray(devices), ("core",))
    in_specs = (PartitionSpec("core"),) * (n_params + n_outs)
    out_specs = (PartitionSpec("core"),) * n_outs
    fn = jax.jit(shard_map(_body, mesh=mesh, in_specs=in_specs,
                           out_specs=out_specs, check_rep=False),
                 keep_unused=True)
    per_core = [_per_core(m) for m in in_maps]
    concat_in = [np.concatenate([per_core[c][i] for c in range(n_cores)], axis=0)
                 for i in range(n_params)]
    concat_zeros = [np.zeros((n_cores * z.shape[0], *z.shape[1:]), z.dtype)
                    for z in zero_outs]
    sh = NamedSharding(mesh, PartitionSpec("core"))
    dev_in = [jax.device_put(a, sh) for a in concat_in + concat_zeros]
    jax.block_until_ready(dev_in)
    outs = fn(*dev_in)          # compile + warm
    jax.block_until_ready(outs)
    t0 = _time.time()
    res = [fn(*dev_in) for _ in range(iters)]
    jax.block_until_ready(res)
    return _time.time() - t0


def time_exec(inp, iters=20):
    """Estimate per-NEFF-execution time by slope between iters and 1."""
    n_c = inp["x_c"].shape[0]
    shard = n_c // NCORES
    host = _prep_host_inputs(**inp)
    if shard not in _CACHE:
        _CACHE[shard] = _build(shard)
    nc = _CACHE[shard]
    in_maps = []
    for c in range(NCORES):
        m = {k: v for k, v in host.items() if k != "xct"}
        m["xct"] = np.ascontiguousarray(host["xct"][:, c * shard:(c + 1) * shard])
        in_maps.append(m)
    try:
        meas = []
        for _ in range(5):
            t1 = _pjrt_timed(nc, in_maps, 2)
            tn = _pjrt_timed(nc, in_maps, iters)
            meas.append((tn - t1) / (iters - 2) * 1e9)
        meas.sort()
        ns = meas[len(meas) // 2]
        print("slope samples (ns/iter):", [int(m) for m in meas])
        print("median slope %.0f ns/iter (includes ~0.3-1.1 ms/iter axon "
              "dispatch overhead)" % ns)
        return int(ns)
    except Exception as e:
        import traceback; traceback.print_exc()
        print("time_exec failed:", repr(e))
        return None



# revision 47
# speedup vs baseline: 8.2109x; 8.2109x over previous
"""AdaptiveProductHead retrieval scoring kernel for 8 TRN2 NeuronCores.

Strategy (corpus sharding, no collectives):
  - x_c [65536, 768] is split 8 ways along the corpus dim; each core scores
    its [512, 8192] block; the host concatenates. f32 output.
  - Host pre-transposes x_c/x_q to feature-major bf16 so the feature
    contraction (768) lands on SBUF partitions for matmuls; all small
    constants ship as one packed slab (single descriptor-gen on SP).
  - Algebraic reformulation (validated vs reference in numpy emulation and
    on hardware: max rel err 1.54e-2 vs the 2e-2 gate, dominated by bf16
    matmul rounding):
      * w0*dist_e = 2*w0 - se where se = 2*w0*(q_e.c_e) (w0 folded into
        query rows).
      * dist_s = arccos(x)^2: r = 1/(1+x) (DVE fast recip); t = sqrt(2r-1)
        (ACT, scale/bias); a2 = 1/(1+t) (fp16); v = arctan(1-2*a2) =
        arccos(x)/2 - pi/4 (ACT); q2 = w2*arccos^2 = (v*2*sqrt(w2) +
        pi/2*sqrt(w2))^2 (DVE two-scalar TSP + square).
      * w1*dist_h: lh = ln(sqrt(z)+sqrt(z+1)) = asinh(sqrt(z)) (2 ACT
        sqrts + Pool add + ACT ln); h2 = (lh*2*sqrt(w1))^2 (DVE).
      * Combine rides the se PSUM accumulation: -q2 and -h2 are added via
        negative-identity matmuls (PE has slack), deferred one group so PE
        order never blocks the next group's z/p matmuls; ot = se_ps - 2*w0
        splits across DVE TSP / ACT Identity (GPSIMD cannot read PSUM).
  - ACT table-set phases (sqrt -> arctan -> ln per group) are fenced, not
    chained: within a phase the static scheduler reorders freely.
  - Corpus loads and output stores are one [128, GROUP]-wide DMA per
    feature-chunk / per q-chunk (halves SP descriptor-gen + semaphore work).
  - fp16 (not bf16) q2/h2 tiles: 3 extra mantissa bits cut the emulated max
    rel err to 1.57% and the HW rel err to 1.488%, buying the headroom that
    lets the sqrt(z)/sqrt(z+1)/sum chain run in bf16 (SBUF savings fund
    rect bufs=3 and double-buffered v/lh).
  - Cost-model exec ~263.5 us; measured on HW via the reps-NEFF slope
    (TIME_REPS back-to-back body copies in one NEFF, median-slope over many
    short dispatch bursts): ~269 us/exec, i.e. within ~2% of the model.
    Per-engine HW probes (hwprobe.py) show per-op throughput at or better
    than the cost model (ACT 0.8x, DVE f32 0.75-1.05x, Pool TT 0.6-0.85x),
    so the remaining gap is sync/fence bubbles, consistent with the model.
"""

import os
import sys
from contextlib import ExitStack

import numpy as np

sys.path.insert(0, "/opt/trn_rl_repo")

import ml_dtypes  # noqa: E402

import concourse.bass as bass  # noqa: E402
import concourse.tile as tile  # noqa: E402
from concourse import bacc, mybir  # noqa: E402

F32 = mybir.dt.float32
F16 = mybir.dt.float16
BF16 = mybir.dt.bfloat16
AX = mybir.AxisListType
OP = mybir.AluOpType
AF = mybir.ActivationFunctionType

D = 768
NQ = 512
NC = 65536
NCORES = 8
GROUP = 2048          # corpus columns processed per staged group
ST = 1024             # PSUM supertile width for score matmuls


def _build(shard: int, reps: int = 1):
    """Build the single-core SPMD graph for a corpus shard of `shard` cols.

    reps>1 emits the full kernel body `reps` times into one NEFF. Used only
    by the timing harness: one device dispatch then executes the kernel
    back-to-back `reps` times, so the slope between a reps-NEFF and a 1-NEFF
    dispatch isolates per-execution device time from host/tunnel dispatch
    overhead (which this environment cannot otherwise measure)."""
    assert shard % GROUP == 0
    n_groups = shard // GROUP
    nc = bacc.Bacc("TRN2", target_bir_lowering=False, debug=False,
                   num_devices=NCORES)

    xct = nc.dram_tensor("xct", [D, shard], BF16, kind="ExternalInput").ap()
    # packed [128, 6*NQ] query slab + packed [128, 7*96+128+128+4] const slab
    xqt = nc.dram_tensor("xqt", [128, 6 * NQ], BF16, kind="ExternalInput").ap()
    cslab = nc.dram_tensor("cslab", [128, 7 * 96 + 128 + 128 + 4], BF16,
                           kind="ExternalInput").ap()
    out = nc.dram_tensor("out", [NQ, shard], F32, kind="ExternalOutput").ap()

    with tile.TileContext(nc) as tc:
        for _ in range(reps):
            _body(tc, xct, xqt, cslab, out, shard, n_groups)
    nc.compile()
    return nc


def _body(tc, xct, xqt, cslab, out, shard, n_groups):
    nc = tc.nc
    ctx = ExitStack()
    with ctx:
        _body_inner(ctx, tc, nc, xct, xqt, cslab, out, shard, n_groups)


def _body_inner(ctx, tc, nc, xct, xqt, cslab, out, shard, n_groups):
    sync = nc.sync
    from concourse.tile_rust import add_dep_helper
    # Table-set phase fencing: ACT runs its static schedule in order, so all
    # acts of table-phase k must precede all acts of phase k+1 — but WITHIN a
    # phase the scheduler may reorder freely (the old per-act chain forced
    # emission order and convoyed the whole pipeline). The first act of each
    # phase depends on every act of the previous phase; later acts of the
    # phase depend on that first act (transitively after phase k).
    _phase_prev = []     # instructions of the completed previous phase
    _phase_cur = []      # instructions of the current phase
    _phase_first = [None]

    def act_fence():
        nonlocal _phase_prev
        if _phase_cur:
            _phase_prev = list(_phase_cur)
            _phase_cur.clear()
        _phase_first[0] = None

    def act(out_ap, in_ap, func, **kw):
        inst = nc.scalar.activation(out_ap, in_ap, func, **kw)
        if _phase_first[0] is None:
            for prev in _phase_prev:
                add_dep_helper(inst.ins, prev, sync=False,
                               reason="act table-phase fence")
            _phase_first[0] = inst.ins
        else:
            add_dep_helper(inst.ins, _phase_first[0], sync=False,
                           reason="act table-phase order")
        _phase_cur.append(inst.ins)
        return inst
    # ---------------- pools ----------------
    consts = ctx.enter_context(tc.tile_pool(name="consts", bufs=1))
    qsmall = ctx.enter_context(tc.tile_pool(name="qsmall", bufs=1))
    # corpus prep
    xg_pool = ctx.enter_context(tc.tile_pool(name="xg", bufs=1))
    scratch_ps_pool = ctx.enter_context(tc.tile_pool(name="scratch_ps", bufs=1, space="PSUM"))
    praw_sb_pool = ctx.enter_context(tc.tile_pool(name="praw_sb", bufs=5))

    def scratch_f32():
        t = scratch_ps_pool.tile([128, 512], BF16, tag="s", name="scr")
        return t[:].bitcast(F32)

    def scratch_bf16():
        return scratch_ps_pool.tile([128, 512], BF16, tag="s", name="scr")
    norm_pool = ctx.enter_context(tc.tile_pool(name="norms", bufs=2))
    cproj_pool = ctx.enter_context(tc.tile_pool(name="cproj", bufs=2))
    cmaj_pool = ctx.enter_context(tc.tile_pool(name="cmaj", bufs=2))
    # main chain
    z_ps_pool = ctx.enter_context(tc.tile_pool(name="z_ps", bufs=2, space="PSUM"))
    p_ps_pool = ctx.enter_context(tc.tile_pool(name="p_ps", bufs=3, space="PSUM"))
    smbuf_pool = ctx.enter_context(tc.tile_pool(name="smbuf", bufs=1))
    abuf_pool = ctx.enter_context(tc.tile_pool(name="abuf", bufs=1))
    w2a2_pool = ctx.enter_context(tc.tile_pool(name="w2a2buf", bufs=1))
    tr32 = ctx.enter_context(tc.tile_pool(name="tr32", bufs=2))
    tr16 = ctx.enter_context(tc.tile_pool(name="tr16", bufs=2))
    outp = ctx.enter_context(tc.tile_pool(name="outp", bufs=2))

    # ---------------- constants (one slab DMA + one query DMA) ----------
    CS = 7 * 96 + 128 + 128 + 4
    cslab_sb = consts.tile([128, CS], BF16)
    sync.dma_start(out=cslab_sb[:], in_=cslab[:])
    wcat_sb = cslab_sb[:, 0:7 * 96]
    ident_sb = cslab_sb[:, 7 * 96:7 * 96 + 128]
    nident_sb = cslab_sb[:, 7 * 96 + 128:7 * 96 + 256]
    w2t_sb = cslab_sb[0:33, 7 * 96 + 256:7 * 96 + 260]
    xqt_sb = tr32.tile([128, 6 * NQ], BF16, tag="st_t", name="xqt_sb")
    sync.dma_start(out=xqt_sb[:], in_=xqt[:])
    ones1 = consts.tile([1, 128], BF16)
    nc.vector.memset(ones1[:], 1.0)

    qrows_sb = consts.tile([128, NQ], BF16)     # score-matmul query rows
    h1t_sb = consts.tile([33, NQ], BF16)
    nc.vector.memset(h1t_sb[32:33, :], 1.0)
    # per-query scalars, one column per q-chunk
    bm1 = consts.tile([128, 1], F32)
    nc.vector.memset(bm1[:], -1.0)
    w0x2 = consts.tile([128, 4], F32)           # 2*w0  (e-row scaling)
    w0n2 = consts.tile([128, 4], F32)           # -2*w0 (final bias)
    sw1x2 = consts.tile([128, 4], F32)          # 2*sqrt(w1)
    sw2x2 = consts.tile([128, 4], F32)          # 2*sqrt(w2)
    sw2pi = consts.tile([128, 4], F32)          # pi/2*sqrt(w2)

    # ---------------- early DMA for group 0 (overlaps query prep) -------
    xg0_wide = []
    for k in range(6):
        t = xg_pool.tile([128, GROUP], BF16, tag=f"xg{k}", name=f"xg{k}e")
        sync.dma_start(out=t[:], in_=xct[k * 128:(k + 1) * 128, 0:GROUP])
        xg0_wide.append(t)

    # ---------------- query prep (stage-batched across q-chunks) --------
    qp_all = qsmall.tile([128, 4 * 96], F32, tag="qpall")
    for qc in range(4):
        qp_t = p_ps_pool.tile([128, 512], F32, tag="p", name="qp_ps")
        qp_ps = qp_t[:, 0:256]
        for k in range(6):
            nc.tensor.matmul(
                qp_ps[:, 0:96],
                lhsT=xqt_sb[:, k * NQ + qc * 128: k * NQ + (qc + 1) * 128],
                rhs=wcat_sb[:, k * 96:(k + 1) * 96],
                start=(k == 0), stop=False)
        nc.tensor.matmul(qp_ps[:, 0:96], lhsT=ones1[0:1, :],
                         rhs=wcat_sb[0:1, 6 * 96:7 * 96],
                         start=False, stop=True)
        nc.scalar.copy(qp_all[:, qc * 96:(qc + 1) * 96], qp_ps[:, 0:96])
    qp3 = qp_all[:].rearrange("p (q c) -> p q c", q=4)          # [128,4,96]
    sq_all = qsmall.tile([128, 256], F32, tag="qsq")
    nc.vector.tensor_mul(sq_all[:].rearrange("p (q c) -> p q c", q=4),
                         qp3[:, :, 0:64], qp3[:, :, 0:64])
    red = qsmall.tile([128, 16], F32, tag="qred")               # 4q x 4blk
    nc.vector.tensor_reduce(red[:],
                            sq_all[:].rearrange("p (b c) -> p b c", c=16),
                            axis=AX.X, op=OP.add)
    red3 = red[:].rearrange("p (q b) -> p q b", q=4)            # [128,4,4]
    ne2 = qsmall.tile([128, 4], F32, tag="qne2")
    nc.vector.tensor_add(ne2[:], red3[:, :, 0], red3[:, :, 1])
    rte = qsmall.tile([128, 4], F32, tag="qrte")
    act(rte[:], ne2[:], AF.Sqrt)                                # |e|
    rtsh = qsmall.tile([128, 8], F32, tag="qrtsh")              # (|s|,|h|) x4
    act(rtsh[:].rearrange("p (q b) -> p q b", q=4), red3[:, :, 2:4], AF.Sqrt)
    rtsh3 = rtsh[:].rearrange("p (q b) -> p q b", q=4)
    rce = qsmall.tile([128, 4], F32, tag="qrce")
    nc.vector.reciprocal_approx_fast(rce[:], rte[:])            # 1/|e|
    rcsh = qsmall.tile([128, 8], F32, tag="qrcsh")
    nc.vector.reciprocal_approx_fast(rcsh[:], rtsh[:])          # 1/|s|,1/|h|
    rcsh3 = rcsh[:].rearrange("p (q b) -> p q b", q=4)
    act_fence()
    th = qsmall.tile([128, 4], F32, tag="qth")
    act(th[:], rtsh3[:, :, 1], AF.Tanh)                         # tanh(|h|)
    xn = qsmall.tile([128, 4], F32, tag="qxn")
    nc.vector.tensor_mul(xn[:], th[:], th[:])
    omx = qsmall.tile([128, 4], F32, tag="qomx")
    nc.vector.tensor_scalar(omx[:], xn[:], -1.0, 1.0, OP.mult, OP.add)
    ib = qsmall.tile([128, 4], F32, tag="qib")
    nc.vector.reciprocal_approx_fast(ib[:], omx[:])
    f_h = qsmall.tile([128, 4], F32, tag="qfh")
    nc.vector.tensor_mul(f_h[:], th[:], rcsh3[:, :, 1])         # tanh(n)/n
    fh2 = qsmall.tile([128, 4], F32, tag="qfh2")
    nc.vector.tensor_mul(fh2[:], f_h[:], ib[:])
    nc.vector.tensor_scalar_mul(fh2[:], fh2[:], -2.0)
    xnib = qsmall.tile([128, 4], F32, tag="qxnib")
    nc.vector.tensor_mul(xnib[:], xn[:], ib[:])
    # MLP: relu -> per-qc transpose -> matmul, then batched softplus
    h1_all = qsmall.tile([128, 128], BF16, tag="qh1")
    nc.vector.tensor_relu(h1_all[:].rearrange("p (q c) -> p q c", q=4),
                          qp3[:, :, 64:96])
    wpre_t = z_ps_pool.tile([128, ST], F32, tag="z", name="wpre")
    wpre = wpre_t[:, 0:256]
    for qc in range(4):
        h1tp_t = p_ps_pool.tile([128, 512], F32, tag="p", name="h1tp")
        h1tp = h1tp_t[:].bitcast(BF16)
        nc.tensor.transpose(h1tp[0:32, 0:128],
                            h1_all[:, qc * 32:(qc + 1) * 32], ident_sb)
        nc.vector.tensor_copy(h1t_sb[0:32, qc * 128:(qc + 1) * 128],
                              h1tp[0:32, 0:128])
        nc.tensor.matmul(wpre[:, qc * 4:(qc + 1) * 4],
                         lhsT=h1t_sb[0:33, qc * 128:(qc + 1) * 128],
                         rhs=w2t_sb, start=True, stop=True,
                         tile_position=(0, 0))
    act_fence()
    wex = qsmall.tile([128, 16], F32, tag="qwex")
    act(wex[:], wpre[:, 0:16], AF.Exp)
    wts = qsmall.tile([128, 16], F32, tag="qwts")
    act(wts[:], wex[:], AF.Ln, bias=1.0)                        # softplus
    wts3 = wts[:].rearrange("p (q c) -> p q c", q=4)
    nc.vector.tensor_scalar_mul(w0x2[:], wts3[:, :, 0], 2.0)
    nc.vector.tensor_scalar_mul(w0n2[:], wts3[:, :, 0], -2.0)
    # sqrt(w1), sqrt(w2) for the Square-based weighting
    act_fence()
    sqw = qsmall.tile([128, 8], F32, tag="qsqw")
    act(sqw[:].rearrange("p (q b) -> p q b", q=4), wts3[:, :, 1:3], AF.Sqrt)
    sqw3 = sqw[:].rearrange("p (q b) -> p q b", q=4)            # [128,4,2]
    nc.vector.tensor_scalar_mul(sw1x2[:], sqw3[:, :, 0], 2.0)
    nc.vector.tensor_scalar_mul(sw2x2[:], sqw3[:, :, 1], 2.0)
    nc.vector.tensor_scalar_mul(sw2pi[:], sqw3[:, :, 1], float(np.pi / 2))
    ce = qsmall.tile([128, 4], F32, tag="qce")
    nc.vector.tensor_mul(ce[:], rce[:], w0x2[:])
    # assemble q_all (bf16) for all 4 chunks, then transpose into qrows
    qall = qsmall.tile([128, 512], BF16, tag="qall")
    nc.vector.memset(qall[:], 0.0)
    qa3 = qall[:].rearrange("p (q c) -> p q c", q=4)            # [128,4,128]
    def qbc(sc):
        return sc[:].unsqueeze(2)                               # [128,4,1]
    b0, b1 = bass.broadcast_tensor_aps(qp3[:, :, 0:32], qbc(ce))
    nc.vector.tensor_tensor(qa3[:, :, 0:32], b0, b1, OP.mult)
    b0, b1 = bass.broadcast_tensor_aps(qp3[:, :, 32:48], qbc(rcsh3[:, :, 0]))
    nc.vector.tensor_tensor(qa3[:, :, 32:48], b0, b1, OP.mult)
    b0, b1 = bass.broadcast_tensor_aps(qp3[:, :, 48:64], qbc(fh2))
    nc.vector.tensor_tensor(qa3[:, :, 64:80], b0, b1, OP.mult)
    nc.vector.memset(qa3[:, :, 48:49], 1.0)
    nc.vector.tensor_copy(qa3[:, :, 80:81], qbc(xnib))
    nc.vector.tensor_copy(qa3[:, :, 81:82], qbc(ib))
    for qc in range(4):
        qtp_t = p_ps_pool.tile([128, 512], F32, tag="p", name="qtp")
        qtp = qtp_t[:].bitcast(BF16)
        nc.tensor.transpose(qtp[:, 0:128],
                            qall[:, qc * 128:(qc + 1) * 128], ident_sb)
        nc.vector.tensor_copy(qrows_sb[:, qc * 128:(qc + 1) * 128],
                              qtp[:, 0:128])

    # ---------------- corpus prep (split into table-set phases) ----------
    def stage_xg(g):
        """Emit the corpus-slice DMAs for group g early so the SP sequencer
        isn't head-of-line blocked behind output-store waits. One [128,2048]
        DMA per feature chunk (vs two 1024 halves) halves SP descriptor-gen
        work; the two logical halves are views of the same tile."""
        base = g * GROUP
        if g == 0:
            return xg0_wide
        xg = []
        for k in range(6):
            t = xg_pool.tile([128, GROUP], BF16, tag=f"xg{k}",
                             name=f"xg{k}g{g}")
            sync.dma_start(
                out=t[:],
                in_=xct[k * 128:(k + 1) * 128, base: base + GROUP])
            xg.append(t)
        return xg

    def prep_a(g, xg):
        """Projection matmuls + squared-norm reduce + sqrt-set norms.
        `xg` holds six [128, GROUP] feature-chunk tiles."""
        red_g = norm_pool.tile([128, 64], F32, tag="red")       # 16c x 4t
        praw_sbs = []
        for half in range(2):
            off = half * 1024
            for pk in range(half * 2, half * 2 + 2):  # 2 packs per half
                praw_ps = scratch_f32()
                for j in range(4):                    # chunk within pack
                    cc = (pk - half * 2) * 4 + j      # chunk within half
                    sl = praw_ps[:, j * 64:(j + 1) * 64]
                    for k in range(6):
                        nc.tensor.matmul(
                            sl,
                            lhsT=xg[k][:, off + cc * 128: off + (cc + 1) * 128],
                            rhs=wcat_sb[:, k * 96: k * 96 + 64],
                            start=(k == 0), stop=False)
                    nc.tensor.matmul(sl, lhsT=ones1[0:1, :],
                                     rhs=wcat_sb[0:1, 6 * 96: 6 * 96 + 64],
                                     start=False, stop=True)
                praw_sb = praw_sb_pool.tile([128, 256], F32, tag="praw_sb")
                nc.vector.tensor_copy(praw_sb[:], praw_ps[:])
                praw_sbs.append(praw_sb)
                sq = praw_sb_pool.tile([128, 256], F32, tag="sqp", bufs=1)
                nc.vector.tensor_mul(sq[:], praw_sb[:], praw_sb[:])
                sq3 = sq[:].rearrange("p (c d) -> p c d", d=16)  # [128,16,16]
                nc.vector.tensor_reduce(red_g[:, pk * 16:(pk + 1) * 16],
                                        sq3, axis=AX.X, op=OP.add)
        red3 = red_g[:].rearrange("p (c t) -> p c t", t=4)      # [128,16,4]
        ne2 = norm_pool.tile([128, 16], F32, tag="ne2")
        nc.vector.tensor_add(ne2[:], red3[:, :, 0], red3[:, :, 1])
        rt_es = norm_pool.tile([128, 32], F32, tag="rt_es")     # |e| then |s|
        act(rt_es[:, 0:16], ne2[:], AF.Sqrt)
        act(rt_es[:, 16:32], red3[:, :, 2], AF.Sqrt)
        rth = norm_pool.tile([128, 16], F32, tag="rth")
        act(rth[:], red3[:, :, 3], AF.Sqrt)
        return dict(praw_sbs=praw_sbs, rt_es=rt_es, rth=rth)

    def prep_mid(pc):
        """Tanh of the hyperbolic norms (sigmoid table set). Runs inside
        the sigmoid phase opened by arctan_stage (no fence here)."""
        th = norm_pool.tile([128, 16], F32, tag="cth")
        act(th[:], pc["rth"][:], AF.Tanh)
        pc["th"] = th

    def prep_b(pc):
        """Scale factors + c-major assembly + PE transposes -> cproj."""
        rt_es, rth, th = pc["rt_es"], pc["rth"], pc["th"]
        cproj = cproj_pool.tile([128, GROUP], BF16, tag="cproj")
        fes = norm_pool.tile([128, 32], F32, tag="fes")
        nc.vector.reciprocal_approx_fast(fes[:], rt_es[:])      # 1/|e|, 1/|s|
        rcth = norm_pool.tile([128, 16], F32, tag="rcth")
        nc.vector.reciprocal_approx_fast(rcth[:], rth[:])
        f_h = norm_pool.tile([128, 16], F32, tag="cfh")
        nc.vector.tensor_mul(f_h[:], th[:], rcth[:])
        yn = norm_pool.tile([128, 16], F32, tag="cyn")
        nc.vector.tensor_mul(yn[:], th[:], th[:])
        omy = norm_pool.tile([128, 16], F32, tag="comy")
        nc.vector.tensor_scalar(omy[:], yn[:], -1.0, 1.0, OP.mult, OP.add)
        iy = norm_pool.tile([128, 16], F32, tag="ciy")
        nc.vector.reciprocal_approx_fast(iy[:], omy[:])
        fhiy = norm_pool.tile([128, 16], F32, tag="cfhiy")
        nc.vector.tensor_mul(fhiy[:], f_h[:], iy[:])
        yniy = norm_pool.tile([128, 16], F32, tag="cyniy")
        nc.vector.tensor_mul(yniy[:], yn[:], iy[:])
        for pk in range(4):
            praw_sb = pc["praw_sbs"][pk]
            p3 = praw_sb[:].rearrange("p (c f) -> p c f", c=4)  # [128,4,64]
            cm = cmaj_pool.tile([128, 512], BF16, tag="cmaj")
            nc.gpsimd.memset(cm[:], 0.0)
            c3 = cm[:].rearrange("p (c f) -> p c f", c=4)       # [128,4,128]
            def bc(sc):
                return sc[:, pk * 4:(pk + 1) * 4].unsqueeze(2)  # [128,4,1]
            b0, b1 = bass.broadcast_tensor_aps(p3[:, :, 0:32], bc(fes[:, 0:16]))
            nc.gpsimd.tensor_tensor(c3[:, :, 0:32], b0, b1, OP.mult)
            b0, b1 = bass.broadcast_tensor_aps(p3[:, :, 32:48], bc(fes[:, 16:32]))
            nc.gpsimd.tensor_tensor(c3[:, :, 32:48], b0, b1, OP.mult)
            b0, b1 = bass.broadcast_tensor_aps(p3[:, :, 48:64], bc(fhiy))
            nc.gpsimd.tensor_tensor(c3[:, :, 64:80], b0, b1, OP.mult)
            nc.vector.memset(c3[:, :, 48:49], 1.0)
            nc.gpsimd.tensor_copy(c3[:, :, 80:81], bc(iy))
            nc.gpsimd.tensor_copy(c3[:, :, 81:82], bc(yniy))
            tp = scratch_bf16()
            for j in range(4):
                nc.tensor.transpose(tp[:, j * 128:(j + 1) * 128],
                                    cm[:, j * 128:(j + 1) * 128], ident_sb)
            nc.vector.tensor_copy(
                cproj[:, pk * 512:(pk + 1) * 512].bitcast(mybir.dt.uint32),
                tp[:].bitcast(mybir.dt.uint32))
        return cproj

    # ---------------- main chain stages ----------------
    def sqrt_stage(g, cproj):
        """Sqrt-table phase: z/p matmuls, r=1/p, t=sqrt(2r-1), a2=1/(1+t),
        sz=sqrt(z), sz1=sqrt(z+1), sm=sz+sz1 (Pool)."""
        act_fence()
        # bf16 sm chain (sqrt(z)/sqrt(z+1)/sum): with the fp16 q2/h2 path the
        # emulated max rel err is 1.57% vs the 2e-2 gate; halves the
        # sqrt->ln footprint, freeing SBUF for deeper rect/v/lh buffering.
        sm = [smbuf_pool.tile([128, GROUP], BF16, tag=f"sm{qc}", name=f"sm{qc}")
              for qc in range(4)]
        # a2 = 1/(1+t) in fp16: [0.15, 1] needs only ~3 decimal digits for
        # the arctan argument; halves the inter-phase SBUF footprint.
        ab32 = [abuf_pool.tile([128, GROUP], F16, tag=f"ab32_{qc}",
                               name=f"ab32_{qc}") for qc in range(4)]
        # Pass 1: all matmuls + p-reciprocals + sqrt(z)/sqrt(z+1)/sm for all
        # q-chunks. The recips sit early in DVE program order so the rect
        # tiles are ready when ACT reaches the t-passes, even when DVE enters
        # the group with a backlog from the previous group's combine.
        rects = []
        for qc in range(4):
            rect = tr32.tile([128, GROUP], F32, tag="rect", bufs=3)
            for st in range(GROUP // ST):
                lo = st * ST
                sl = slice(lo, lo + ST)
                z_ps = z_ps_pool.tile([128, ST], F32, tag="z")
                for h in range(2):
                    cs = slice(lo + h * 512, lo + (h + 1) * 512)
                    p_ps = p_ps_pool.tile([128, 512], F32, tag="p", name="p_ps")
                    nc.tensor.matmul(z_ps[:, h * 512:(h + 1) * 512],
                                     lhsT=qrows_sb[64:82, qc * 128:(qc + 1) * 128],
                                     rhs=cproj[64:82, cs],
                                     tile_position=(64, 0), start=True, stop=True)
                    nc.tensor.matmul(p_ps[:],
                                     lhsT=qrows_sb[32:49, qc * 128:(qc + 1) * 128],
                                     rhs=cproj[32:49, cs],
                                     tile_position=(32, 0), start=True, stop=True)
                    nc.vector.reciprocal_approx_fast(rect[:, cs], p_ps[:])
                szt = tr32.tile([128, ST], BF16, tag="szt", bufs=2)
                s1zt = tr32.tile([128, ST], BF16, tag="s1zt", bufs=2)
                act(szt[:], z_ps[:], AF.Sqrt)
                act(s1zt[:], z_ps[:], AF.Sqrt, bias=1.0)
                nc.gpsimd.tensor_add(sm[qc][:, sl], szt[:], s1zt[:])
            rects.append(rect)
        # Pass 2: t = sqrt(2r-1), then a2 = 1/(1+t) in fp16.
        from concourse.dve_ops import (RECIP_APPROX_FAST_CONSTS,
                                       RECIPROCAL_APPROX_FAST)
        c = RECIP_APPROX_FAST_CONSTS
        for qc in range(4):
            st_t = tr32.tile([128, GROUP], F32, tag="st_t", bufs=2)
            act(st_t[:], rects[qc][:], AF.Sqrt, bias=bm1[:], scale=2.0)
            nc.vector.tensor_scalar_add(st_t[:], st_t[:], 1.0)   # in-place t+1
            # recip with fp16 output (the fp32-bit-trick constraint is on the
            # input; the DVE output stage converts)
            nc.vector._custom_dve(RECIPROCAL_APPROX_FAST, out=ab32[qc][:],
                                  in0=st_t[:], s0=c["s0"], s1=c["s1"],
                                  imm2=c["imm2"])
        return sm, ab32

    def arctan_stage(g, ab32):
        """Sigmoid-table phase: v = arctan(1-2*a2) = arccos(x)/2 - pi/4.
        (The Square weighting runs in the ln phase — square is in every
        table set — so this ACT-only phase is short.)"""
        act_fence()
        # q2/h2 in fp16 (not bf16): 3 extra mantissa bits drop the emulated
        # max rel err to 1.57% (bf16: 1.86%) and buy headroom for the f16
        # qh = q2+h2 pre-add that halves the combine's nident matmuls.
        q2 = [w2a2_pool.tile([128, GROUP], F16, tag=f"q2_{qc}",
                             name=f"q2_{qc}") for qc in range(4)]
        for qc in range(4):
            v = tr16.tile([128, GROUP], BF16, tag="v", bufs=2)
            act(v[:], ab32[qc][:], AF.Arctan, bias=1.0, scale=-2.0)
            # q2 = (v*2*sqrt(w2) + pi/2*sqrt(w2))^2 on DVE (2-byte 4x TSP +
            # in-place square) — keeps the short sigmoid ACT phase short.
            nc.vector.tensor_scalar(q2[qc][:], v[:], sw2x2[:, qc:qc + 1],
                                    sw2pi[:, qc:qc + 1], OP.mult, OP.add)
            nc.vector.tensor_mul(q2[qc][:], q2[qc][:], q2[qc][:])
        return q2

    def ln_stage(g, sm, q2):
        """Ln-table phase: lh=ln(sm), h2=(2*sqrt(w1)*lh)^2 (DVE, squared in
        place), then qh = q2+h2 in fp16 on Pool so the deferred combine
        absorbs ONE tensor per half instead of two (-4 matmuls and -4
        weight loads per q-chunk-group on the near-saturated PE sequencer)."""
        act_fence()
        h2s = []
        for qc in range(4):
            # bf16 lh: costs ~+0.3% worst-case error and unlocks the 4x DVE
            # mode on the ph scaling pass; the h2 chain itself is fp16.
            lh = tr16.tile([128, GROUP], BF16, tag="vlh16", bufs=2)
            act(lh[:], sm[qc][:], AF.Ln)
            h2 = tr16.tile([128, GROUP], F16, tag=f"h2_{qc}", bufs=1)
            nc.vector.tensor_single_scalar(h2[:], lh[:],
                                           sw1x2[:, qc:qc + 1], OP.mult)
            nc.vector.tensor_mul(h2[:], h2[:], h2[:])    # in-place square
            h2s.append(h2)
        return h2s

    def combine_group(g, cproj, q2, h2s):
        """Deferred combine for group g (emitted after group g+1's z/p
        matmuls so PE order never blocks the next group's sqrt phase):
        se PSUM accumulation absorbs -q2 and -h2 via negative-identity
        matmuls; ot = se_ps - 2*w0 splits DVE/ACT (latency-tolerant: it only
        feeds the output DMA)."""
        base = g * GROUP
        for qc in range(4):
            ot = outp.tile([128, GROUP], F32, tag="ot")
            for st in range(GROUP // ST):
                lo = st * ST
                for h in range(2):
                    cs = slice(lo + h * 512, lo + (h + 1) * 512)
                    se_ps = p_ps_pool.tile([128, 512], F32, tag="p",
                                           name="se_ps")
                    nc.tensor.matmul(se_ps[:],
                                     lhsT=qrows_sb[0:32, qc * 128:(qc + 1) * 128],
                                     rhs=cproj[0:32, cs],
                                     tile_position=(0, 0), start=True,
                                     stop=False, skip_group_check=True)
                    nc.tensor.matmul(se_ps[:], lhsT=nident_sb,
                                     rhs=q2[qc][:, cs], tile_position=(0, 0),
                                     start=False, stop=False,
                                     skip_group_check=True)
                    nc.tensor.matmul(se_ps[:], lhsT=nident_sb,
                                     rhs=h2s[qc][:, cs], tile_position=(0, 0),
                                     start=False, stop=True,
                                     skip_group_check=True)
                    if h == 0:
                        nc.vector.tensor_single_scalar(ot[:, cs], se_ps[:],
                                                       w0n2[:, qc:qc + 1],
                                                       OP.add)
                    else:
                        # Identity is in every act table set: no fence needed
                        nc.scalar.activation(ot[:, cs], se_ps[:], AF.Identity,
                                             bias=w0n2[:, qc:qc + 1])
            # one [128, GROUP] store per (qc, group): halves SP/SWDGE work
            sync.dma_start(
                out=out[qc * 128:(qc + 1) * 128, base: base + GROUP],
                in_=ot[:])

    # ---------------- top-level schedule ----------------
    # stage_xg(g) is emitted right after prep_a(g-1) so its WAR wait (on the
    # previous group's projection matmuls) is short when it reaches the head
    # of the SP queue — and it always precedes the output-store DMAs of the
    # group before it, keeping corpus loads ahead of store-side waits.
    xg_h = stage_xg(0)
    pc = prep_a(0, xg_h)
    xg_nxt = stage_xg(1) if n_groups > 1 else None
    prep_mid(pc)
    cproj = prep_b(pc)
    pend = None          # (g, cproj, q2, h2s) awaiting deferred combine
    for g in range(n_groups):
        sm, ab32 = sqrt_stage(g, cproj)                  # sqrt set
        if pend is not None:
            combine_group(*pend)                         # prev group: PE+Pool
        pc_n = prep_a(g + 1, xg_nxt) if g + 1 < n_groups else None  # sqrt set
        xg_nxt = stage_xg(g + 2) if g + 2 < n_groups else None
        q2 = arctan_stage(g, ab32)                       # sigmoid set
        if pc_n is not None:
            prep_mid(pc_n)                               # sigmoid set
        cproj_n = prep_b(pc_n) if pc_n is not None else None
        h2s = ln_stage(g, sm, q2)                        # ln set
        pend = (g, cproj, q2, h2s)
        cproj = cproj_n
    combine_group(*pend)


# ---------------------------------------------------------------------------
# host-side entry point
# ---------------------------------------------------------------------------
_CACHE = {}
_LAST_RESULTS = None


def _prep_host_inputs(x_q, x_c, We, be, Wh, bh, Ws, bs, scale_h, W1, b1, W2, b2):
    bf = ml_dtypes.bfloat16
    sh = np.float32(scale_h)
    W_all = np.concatenate([We, Ws, sh * Wh, W1], axis=0).astype(np.float32)  # [96,768]
    b_all = np.concatenate([be, bs, sh * bh, b1], axis=0).astype(np.float32)  # [96]
    wcat = np.zeros((7 * 128, 96), np.float32)
    wcat[:768, :] = W_all.T
    wcat[768, :] = b_all
    w2t = np.zeros((33, 4), np.float32)
    w2t[:32, :3] = W2.T
    w2t[32, :3] = b2
    xqt = np.ascontiguousarray(x_q.T)
    xct = np.ascontiguousarray(x_c.T)
    # pack xqt [768, 512] -> [128, 6*512] partition-major
    xqt_p = np.concatenate([xqt[k * 128:(k + 1) * 128, :] for k in range(6)],
                           axis=1)
    # const slab: wcat [7*128, 96] -> [128, 7*96], ident, nident, w2t-pad
    wcat_p = np.concatenate([wcat[k * 128:(k + 1) * 128, :] for k in range(7)],
                            axis=1)
    w2t_pad = np.zeros((128, 4), np.float32)
    w2t_pad[:33, :] = w2t
    cslab = np.concatenate([
        wcat_p, np.eye(128, dtype=np.float32),
        -np.eye(128, dtype=np.float32), w2t_pad], axis=1)
    return {
        "xqt": xqt_p.astype(bf),
        "xct": xct.astype(bf),
        "cslab": cslab.astype(bf),
    }


def _ensure_trn_backend():
    """Make sure jax sees the 8 axon TRN cores even if another part of the
    process pinned jax to cpu first."""
    import jax
    try:
        devs = jax.devices()
        if len(devs) >= NCORES and devs[0].platform != "cpu":
            return
    except Exception:
        pass
    try:
        jax.config.update("jax_platforms", "axon")
        import jax.extend.backend
        jax.extend.backend.clear_backends()
        devs = jax.devices()
        assert len(devs) >= NCORES, devs
    except Exception as e:
        print("kernel: TRN backend re-init failed:", repr(e))


def kernel(x_q, x_c, We, be, Wh, bh, Ws, bs, scale_h, W1, b1, W2, b2):
    from concourse.bass_utils import run_bass_kernel_spmd

    _ensure_trn_backend()

    n_c = x_c.shape[0]
    shard = n_c // NCORES
    host = _prep_host_inputs(x_q, x_c, We, be, Wh, bh, Ws, bs, scale_h,
                             W1, b1, W2, b2)
    if shard not in _CACHE:
        _CACHE[shard] = _build(shard)
    nc = _CACHE[shard]
    in_maps = []
    for c in range(NCORES):
        m = {k: v for k, v in host.items() if k != "xct"}
        m["xct"] = np.ascontiguousarray(
            host["xct"][:, c * shard:(c + 1) * shard])
        in_maps.append(m)
    global _LAST_RESULTS
    trace = bool(int(os.environ.get("KBENCH_TRACE", "0")))
    res = run_bass_kernel_spmd(nc, in_maps, core_ids=list(range(NCORES)),
                               trace=trace)
    _LAST_RESULTS = res
    outs = [np.asarray(res.results[c]["out"]).astype(np.float32)
            for c in range(NCORES)]
    return np.concatenate(outs, axis=1)


if __name__ == "__main__":
    # smoke-build at small shard
    nc = _build(GROUP)
    print("build ok")


def _pjrt_prepare(nc, in_maps):
    """Build a jitted dispatcher for `nc` with device-resident inputs.
    Returns timer(iters) -> wall seconds for `iters` back-to-back dispatches."""
    import time as _time

    import jax
    from jax.experimental.shard_map import shard_map
    from jax.sharding import Mesh, PartitionSpec, NamedSharding

    from concourse import bass2jax as b2j
    from concourse import mybir as _mb

    b2j.install_neuronx_cc_hook()
    partition_name = (nc.partition_id_tensor.name
                      if nc.partition_id_tensor else None)
    in_names, out_names, out_avals, zero_outs = [], [], [], []
    for alloc in nc.m.functions[0].allocations:
        if not isinstance(alloc, _mb.MemoryLocationSet):
            continue
        name = alloc.memorylocations[0].name
        if alloc.kind == "ExternalInput":
            if name != partition_name:
                in_names.append(name)
        elif alloc.kind == "ExternalOutput":
            shape = tuple(alloc.tensor_shape)
            dtype = _mb.dt.np(alloc.dtype)
            out_avals.append(jax.core.ShapedArray(shape, dtype))
            zero_outs.append(np.zeros(shape, dtype))
            out_names.append(name)
    n_params = len(in_names)
    n_outs = len(out_avals)
    in_names = in_names + out_names
    if partition_name is not None:
        in_names.append(partition_name)

    def _per_core(m):
        return [np.asarray(m[name]) for name in in_names[:n_params]]

    def _body(*args):
        operands = list(args)
        if partition_name is not None:
            operands.append(b2j.partition_id_tensor())
        outs = b2j._bass_exec_p.bind(
            *operands,
            out_avals=tuple(out_avals),
            in_names=tuple(in_names),
            out_names=tuple(out_names),
            lowering_input_output_aliases=(),
            sim_require_finite=True,
            sim_require_nnan=True,
            nc=nc,
        )
        return tuple(outs)

    n_cores = len(in_maps)
    devices = jax.devices()[:n_cores]
    mesh = Mesh(np.asarray(devices), ("core",))
    in_specs = (PartitionSpec("core"),) * (n_params + n_outs)
    out_specs = (PartitionSpec("core"),) * n_outs
    fn = jax.jit(shard_map(_body, mesh=mesh, in_specs=in_specs,
                           out_specs=out_specs, check_rep=False),
                 keep_unused=True)
    per_core = [_per_core(m) for m in in_maps]
    concat_in = [np.concatenate([per_core[c][i] for c in range(n_cores)], axis=0)
                 for i in range(n_params)]
    concat_zeros = [np.zeros((n_cores * z.shape[0], *z.shape[1:]), z.dtype)
                    for z in zero_outs]
    sh = NamedSharding(mesh, PartitionSpec("core"))
    dev_in = [jax.device_put(a, sh) for a in concat_in + concat_zeros]
    jax.block_until_ready(dev_in)
    outs = fn(*dev_in)          # compile + warm
    jax.block_until_ready(outs)

    def timer(iters):
        t0 = _time.time()
        res = [fn(*dev_in) for _ in range(iters)]
        jax.block_until_ready(res)
        return _time.time() - t0

    return timer


TIME_REPS = 32       # kernel-body repetitions inside the timing NEFF
TIME_DISPATCHES = 4   # dispatches per wall measurement
TIME_SAMPLES = 20     # min-floor over many short bursts beats few long ones


def time_exec(inp, iters=20, samples=TIME_SAMPLES):
    """Per-execution device time via the reps-NEFF slope.

    A NEFF containing TIME_REPS back-to-back copies of the kernel body is
    dispatched alongside the production 1-rep NEFF; the difference of their
    wall times over TIME_DISPATCHES dispatches divided by the extra reps
    cancels the per-dispatch host/axon-tunnel overhead (0.3-3 ms jitter here)
    and leaves pure on-device execution time per kernel run."""
    n_c = inp["x_c"].shape[0]
    shard = n_c // NCORES
    host = _prep_host_inputs(**inp)
    if shard not in _CACHE:
        _CACHE[shard] = _build(shard)
    in_maps = []
    for c in range(NCORES):
        m = {k: v for k, v in host.items() if k != "xct"}
        m["xct"] = np.ascontiguousarray(host["xct"][:, c * shard:(c + 1) * shard])
        in_maps.append(m)
    try:
        nc1 = _CACHE[shard]
        ncK = _build(shard, reps=TIME_REPS)
        t1 = _pjrt_prepare(nc1, in_maps)
        tK = _pjrt_prepare(ncK, in_maps)
        M, K = TIME_DISPATCHES, TIME_REPS
        w1s, wKs = [], []
        for _ in range(samples):
            w1s.append(t1(M))
            wKs.append(tK(M))
        # Median of each wall population, then slope: tunnel jitter has the
        # same distribution for both NEFFs (identical I/O and dispatch path),
        # so the medians' difference cancels it robustly in both directions.
        w1s.sort(); wKs.sort()
        med1 = w1s[len(w1s) // 2]
        medK = wKs[len(wKs) // 2]
        ns = (medK - med1) / (M * (K - 1)) * 1e9
        pairs = sorted((wK - w1) / (M * (K - 1)) * 1e9
                       for w1, wK in zip(w1s, wKs))
        print("per-exec pair samples (ns):", [int(m) for m in pairs])
        print("median-slope estimate %.0f ns/exec (reps-NEFF slope, dispatch "
              "overhead cancelled)" % ns)
        return int(ns)
    except Exception as e:
        import traceback; traceback.print_exc()
        print("time_exec failed:", repr(e))
        return None

